# revision 9
# baseline (speedup 1.0000x reference)
"""Dice-loss-by-block kernel for Trainium2 (8 NeuronCores, batch-parallel).

Algorithm (per core = one batch element, data viewed as [128, 16384]):
  Per-label sums S_l[v] = sum(v * [s == l]) for v in {x, t, x*t}, l = 1..10
  via the ramp identity  R_l = sum(relu(u - l)), u = s + v, v in [0,1):
  S_l[v] = R_l - R_{l+1} - C_{>=l+1}, with exact counts C from host bincount.

  30 ramp functionals per 4096-col super-chunk, spread over every piece of
  silicon that can reduce (v5, HW-trace calibrated):
    * builds (all fp16, all on DVE at 2x; GPSIMD tensor ops are 4-5x
      slower AND poison DVE via the shared SBUF port): s16/t16/x16 casts,
      u_t, u_x, xt, u_xt -- ~8.4us per 2048-col chunk.
    * ACT path (10): fused relu+accum at 1x (~4.0us/func).
    * fold path (15): DVE UNFUSED tensor_scalar(max) at 4x (~1.2us) ->
      fp16 scratch; an ACCUMULATING DMA (SWDGE via idle GPSIMD; the SDMA
      CCE does dst += src) folds scr[0:2048] += scr[2048:4096]; TensorE
      then reduces 2048 cols via 4 selector-matmuls into PSUM row j.
      DMA engines are ~13% busy otherwise -- this offloads half the
      reduction bytes to them.
    * direct path (5): DVE TS(max) -> 8 selector-matmuls.
  PE details: selector = ones column j of a sliding [128,32] window into a
  zeros strip; one PSUM bank accumulates all functionals over the whole
  kernel (start on the very first warmup matmul, stop on the last real
  one).  48 warmup matmuls through an all-zero selector window keep the
  PE HAM clock-gate at 8/8 before real work lands (a LDWEIGHTS between
  accumulating matmuls forces isolated fill+drain, ~414ns per 512-col
  matmul instead of 216).
  Fused/dummy outputs use stride-0 broadcast APs (engines tolerate write
  collisions; saves 24KB SBUF).
  Final: PSUM [32,512] -> SBUF -> DRAM; accums -> DRAM; host recovers R_l,
  applies exact count corrections and the dice formula in float64.
"""

import numpy as np

# ---- hardcoded problem geometry -------------------------------------------
B = 8                      # batch == number of cores
P = 128                    # SBUF partitions
F = 16384                  # free dim per core (128*128*128 / 128)
N = P * F                  # elements per core
NB = 10                    # labels 1..10 (0 = background)
STAGE = 2048               # DMA staging columns
UCOLS = 4096               # u-tile columns per super-chunk
NSUPER = F // UCOLS        # 4
NCH = UCOLS // STAGE       # 2 staging chunks per super
PE_CHUNK = 512             # matmul moving free dim
N_WARMUP = 24              # PE warmup matmuls
EPS = 1e-6

# Functional assignment (fixed across supers).  All PE-path funcs are
# DMA-folded (4096->2048) then reduced by 4 matmuls.  Ordered by stream
# readiness (u_t completes first, then u_x, then u_xt).
PE_FUNCS = (
    [("ut", 10)]
    + [("ux", l) for l in range(1, 11)]
    + [("uxt", l) for l in range(1, 10)]
)  # PSUM row j = index; 20 rows, max-form
ACT_FUNCS = [("ut", l) for l in range(1, 10)] + [("uxt", 10)]  # relu-form
N_EARLY_HALVES = 5         # super-0: first K funcs emitted as two halves
PE_LAG = 2                 # funcs between a fold-DMA and its matmuls

NROW = 32                  # SEL width / psum partition rows

_CACHE = {}


def _build_program():
    import concourse.mybir as mybir
    from concourse import bacc, tile

    fp32 = mybir.dt.float32
    fp16 = mybir.dt.float16
    int32 = mybir.dt.int32
    Alu = mybir.AluOpType
    Act = mybir.ActivationFunctionType

    nc = bacc.Bacc("TRN2", target_bir_lowering=False, debug=False)

    # activation(bias=float) needs a registered const AP per value
    for l in range(1, 11):
        val = float(-l)
        th = nc.alloc_sbuf_tensor(f"const-float32--{l}", [128, 1], fp32)
        nc.gpsimd.memset(th.ap(), val)
        nc.const_aps.aps[(fp32, val)] = th.ap()
    nc.all_engine_barrier()

    x_d = nc.dram_tensor("x", [P, F], fp32, kind="ExternalInput").ap()
    t_d = nc.dram_tensor("t", [P, F], fp32, kind="ExternalInput").ap()
    s_d = nc.dram_tensor("s", [P, F], int32, kind="ExternalInput").ap()

    n_acc_cols = len(ACT_FUNCS) * NSUPER
    acc_d = nc.dram_tensor("acc", [P, n_acc_cols], fp32, kind="ExternalOutput").ap()
    pe_d = nc.dram_tensor("pe", [NROW, PE_CHUNK], fp32, kind="ExternalOutput").ap()

    def ramp_plan(si):
        # entries: (j, kind, l, lo, hi); every entry is TS+fold+(hi-lo)/2/512 MMs
        plan = []
        for j, (kind, l) in enumerate(PE_FUNCS):
            if si == 0 and j < N_EARLY_HALVES:
                plan.append((j, kind, l, 0, UCOLS // 2))
                plan.append((j, kind, l, UCOLS // 2, UCOLS))
            else:
                plan.append((j, kind, l, 0, UCOLS))
        return plan

    n_mm = sum(
        (hi - lo) // 2 // PE_CHUNK
        for si in range(NSUPER)
        for (_, _, _, lo, hi) in ramp_plan(si)
    )

    with tile.TileContext(nc) as tc:
        with (
            tc.tile_pool(name="io", bufs=2) as io_pool,
            tc.tile_pool(name="tr", bufs=2) as tr_pool,
            tc.tile_pool(name="up", bufs=2) as u_pool,
            tc.tile_pool(name="scr", bufs=6) as scr_pool,
            tc.tile_pool(name="persist", bufs=1) as pp,
            tc.tile_pool(name="psum", bufs=1, space="PSUM") as psp,
        ):
            # SEL strip: ones at column 32; SEL_j = strip[:, 32-j : 64-j].
            # strip[:, 0:32] is an all-zero selector (warmup).
            strip = pp.tile([P, 64], fp16, tag="strip")
            nc.vector.memset(strip[:], 0.0)
            nc.vector.memset(strip[:, 32:33], 1.0)

            acc_f = pp.tile([P, n_acc_cols], fp32, tag="acc_f")
            # stride-0 dummy output for fused ACT ramps (never read)
            dummy = pp.tile([P, 1], fp16, tag="dummy")
            dummy_bcast = dummy[:, 0:1].broadcast_to((P, UCOLS))
            scr_w = pp.tile([P, PE_CHUNK], fp16, tag="scr_w")
            nc.vector.memset(scr_w[:], 0.0)
            psum = psp.tile([NROW, PE_CHUNK], fp32, tag="psum")

            # PE warmup: zero-selector matmuls hold the HAM clock warm and
            # initialize PSUM (first has start=True).
            sel0 = strip[:, 0:32]
            for w in range(N_WARMUP):
                nc.tensor.matmul(
                    psum[:], sel0, scr_w[:],
                    start=(w == 0), stop=False, skip_group_check=True,
                )

            mm_idx = 0

            def pe_reduce(j, scr, ncols):
                nonlocal mm_idx
                sel = strip[:, 32 - j : 64 - j]
                for c in range(ncols // PE_CHUNK):
                    mm_idx += 1
                    nc.tensor.matmul(
                        psum[:],
                        sel,
                        scr[:, c * PE_CHUNK : (c + 1) * PE_CHUNK],
                        start=False,
                        stop=(mm_idx == n_mm),
                        skip_group_check=True,
                    )

            acc_col = 0

            def fused_col():
                nonlocal acc_col
                c = acc_col
                acc_col += 1
                return acc_f[:, c : c + 1]

            u_sets = [None] * NSUPER

            def alloc_u(si):
                u_x = u_pool.tile([P, UCOLS], fp16, tag="u_x")
                u_t = u_pool.tile([P, UCOLS], fp16, tag="u_t")
                u_xt = u_pool.tile([P, UCOLS], fp16, tag="u_xt")
                u_sets[si] = {"ux": u_x, "ut": u_t, "uxt": u_xt}

            def build_thunks(si):
                """One thunk = one DVE build op; the first thunk of each
                chunk also issues that chunk's DMAs."""
                srcs = u_sets[si]
                thunks = []
                for c in range(NCH):
                    ci = si * NCH + c
                    sl = slice(ci * STAGE, (ci + 1) * STAGE)
                    hsl = slice(c * STAGE, (c + 1) * STAGE)
                    state = {}

                    def dma_and_s16(sl=sl, state=state):
                        x_c = io_pool.tile([P, STAGE], fp32, tag="x_c")
                        t_c = io_pool.tile([P, STAGE], fp32, tag="t_c")
                        s_c = io_pool.tile([P, STAGE], int32, tag="s_c")
                        nc.sync.dma_start(out=x_c[:], in_=x_d[:, sl])
                        nc.sync.dma_start(out=t_c[:], in_=t_d[:, sl])
                        nc.sync.dma_start(out=s_c[:], in_=s_d[:, sl])
                        s16 = tr_pool.tile([P, STAGE], fp16, tag="s16")
                        t16 = tr_pool.tile([P, STAGE], fp16, tag="t16")
                        x16 = tr_pool.tile([P, STAGE], fp16, tag="x16")
                        xt16 = tr_pool.tile([P, STAGE], fp16, tag="xt16")
                        state.update(x_c=x_c, t_c=t_c, s_c=s_c, s16=s16,
                                     t16=t16, x16=x16, xt16=xt16)
                        nc.vector.tensor_copy(s16[:], s_c[:])

                    thunks.append(dma_and_s16)
                    thunks.append(lambda st=state: nc.vector.tensor_copy(
                        st["t16"][:], st["t_c"][:]))
                    thunks.append(lambda st=state: nc.vector.tensor_copy(
                        st["x16"][:], st["x_c"][:]))
                    thunks.append(lambda st=state, h=hsl: nc.vector.tensor_tensor(
                        srcs["ut"][:, h], st["t16"][:], st["s16"][:], Alu.add))
                    thunks.append(lambda st=state, h=hsl: nc.vector.tensor_tensor(
                        srcs["ux"][:, h], st["x16"][:], st["s16"][:], Alu.add))
                    thunks.append(lambda st=state: nc.vector.tensor_tensor(
                        st["xt16"][:], st["x16"][:], st["t16"][:], Alu.mult))
                    thunks.append(lambda st=state, h=hsl: nc.vector.tensor_tensor(
                        srcs["uxt"][:, h], st["xt16"][:], st["s16"][:], Alu.add))
                return thunks

            pending = []  # lag queue of (j, scr, folded_cols)

            def flush_pending(keep):
                while len(pending) > keep:
                    j, scr, ncols = pending.pop(0)
                    pe_reduce(j, scr, ncols)

            def ramp_thunks(si):
                srcs = u_sets[si]
                thunks = []
                for j, kind, l, lo, hi in ramp_plan(si):
                    def ts(j=j, kind=kind, l=l, lo=lo, hi=hi):
                        w = hi - lo
                        scr = scr_pool.tile([P, w], fp16, tag="scr")
                        nc.vector.tensor_scalar(
                            scr[:], srcs[kind][:, lo:hi], float(l), None,
                            Alu.max
                        )
                        nc.gpsimd.dma_start(
                            out=scr[:, 0 : w // 2],
                            in_=scr[:, w // 2 : w],
                            accum_op=Alu.add,
                        )
                        pending.append((j, scr, w // 2))
                        flush_pending(PE_LAG)
                    thunks.append(ts)
                return thunks

            # super 0 builds upfront
            alloc_u(0)
            for th in build_thunks(0):
                th()

            for si in range(NSUPER):
                srcs = u_sets[si]
                for kind, l in ACT_FUNCS:
                    nc.scalar.activation(
                        dummy_bcast, srcs[kind][:], Act.Relu,
                        bias=float(-l), scale=1.0,
                        accum_out=fused_col(),
                    )
                # DVE: ramps of si interleaved with builds of si+1
                nxt = []
                if si + 1 < NSUPER:
                    alloc_u(si + 1)
                    nxt = build_thunks(si + 1)
                ramps = ramp_thunks(si)
                ri = bi = 0
                while ri < len(ramps) or bi < len(nxt):
                    if ri < len(ramps):
                        ramps[ri]()
                        ri += 1
                    if bi < len(nxt):
                        nxt[bi]()
                        bi += 1

            flush_pending(0)
            assert mm_idx == n_mm
            pe_sb = pp.tile([NROW, PE_CHUNK], fp32, tag="pe_sb")
            nc.vector.tensor_copy(pe_sb[:], psum[:])
            nc.sync.dma_start(out=pe_d[:], in_=pe_sb[:])
            nc.sync.dma_start(out=acc_d[:], in_=acc_f[:])

    nc.compile()
    return nc


def _get_program():
    if "nc" not in _CACHE:
        _CACHE["nc"] = _build_program()
    return _CACHE["nc"]


def _recover_sums(acc, pe, Cge):
    """acc: [P, n_acc_cols] fp32; pe: [NROW, PE_CHUNK] fp32; Cge: exact
    C_{>=l} counts (len 13).  Returns S[kind][l] for l=1..10."""
    R = {v: np.zeros(12) for v in ("ux", "ut", "uxt")}
    accs = acc.astype(np.float64)
    col = 0
    for si in range(NSUPER):
        for kind, l in ACT_FUNCS:       # relu-form: R_l directly
            R[kind][l] += accs[:, col].sum()
            col += 1
    pes = pe.astype(np.float64)
    for j, (kind, l) in enumerate(PE_FUNCS):  # max-form, full stream
        R[kind][l] += pes[j].sum() - l * N

    S = {}
    for v in ("ux", "ut", "uxt"):
        Sv = np.zeros(11)
        for l in range(1, 11):
            Rl1 = R[v][l + 1] if l + 1 <= 10 else 0.0
            Sv[l] = R[v][l] - Rl1 - Cge[l + 1]
        S[v] = Sv
    return S


def kernel(input, target, block):
    from concourse.bass_utils import run_bass_kernel_spmd

    nc = _get_program()

    in_maps = []
    for b in range(B):
        in_maps.append(
            {
                "x": np.ascontiguousarray(input[b].reshape(P, F)),
                "t": np.ascontiguousarray(target[b].reshape(P, F)),
                "s": np.ascontiguousarray(block[b].reshape(P, F)),
            }
        )
    res = run_bass_kernel_spmd(nc, in_maps, list(range(B))).results

    intersect = np.zeros((B, NB))
    input_area = np.zeros((B, NB))
    target_area = np.zeros((B, NB))
    counts = np.zeros((B, NB))
    for b in range(B):
        cnt = np.bincount(block[b].reshape(-1), minlength=12)[:12].astype(np.float64)
        Cge = np.concatenate([np.cumsum(cnt[::-1])[::-1], [0.0]])  # C_{>=l}, l=0..12
        S = _recover_sums(res[b]["acc"], res[b]["pe"], Cge)
        input_area[b] = S["ux"][1:11]
        target_area[b] = S["ut"][1:11]
        intersect[b] = S["uxt"][1:11]
        counts[b] = cnt[1:11]

    # dice combination (mirror reference, float64; empty-segment test uses
    # exact integer counts, equivalent to target_area == 0 for this data)
    empty = counts == 0
    denom = input_area + target_area + 2.0 * EPS
    batch_loss = 1.0 - 2.0 * intersect / denom
    batch_loss = np.where(empty, 0.0, batch_loss)
    valid = (~empty).sum(axis=0).astype(np.float64)
    loss_per_block = batch_loss.sum(axis=0) / np.maximum(valid, 1.0)

    present = counts.sum(axis=0) > 0
    num = present.sum()
    loss = np.where(present, loss_per_block, 0.0).sum() / num
    return (np.float32(loss), 0)


# revision 11
# speedup vs baseline: 1.0693x; 1.0693x over previous
"""Dice-loss-by-block kernel for Trainium2 (8 NeuronCores, batch-parallel).

Algorithm (per core = one batch element, data viewed as [128, 16384]):
  Per-label sums S_l[v] = sum(v * [s == l]) for v in {x, t, x*t}, l = 1..10
  via the ramp identity  R_l = sum(relu(u - l)), u = s + v, v in [0,1):
  S_l[v] = R_l - R_{l+1} - C_{>=l+1}, with exact counts C from host bincount.

  30 ramp functionals per 4096-col super-chunk, spread over every piece of
  silicon that can reduce (v5, HW-trace calibrated):
    * builds (all fp16, all on DVE at 2x; GPSIMD tensor ops are 4-5x
      slower AND poison DVE via the shared SBUF port): s16/t16/x16 casts,
      u_t, u_x, xt, u_xt -- ~8.4us per 2048-col chunk.
    * ACT path (10): fused relu+accum at 1x (~4.0us/func).
    * fold path (15): DVE UNFUSED tensor_scalar(max) at 4x (~1.2us) ->
      fp16 scratch; an ACCUMULATING DMA (SWDGE via idle GPSIMD; the SDMA
      CCE does dst += src) folds scr[0:2048] += scr[2048:4096]; TensorE
      then reduces 2048 cols via 4 selector-matmuls into PSUM row j.
      DMA engines are ~13% busy otherwise -- this offloads half the
      reduction bytes to them.
    * direct path (5): DVE TS(max) -> 8 selector-matmuls.
  PE details: selector = ones column j of a sliding [128,32] window into a
  zeros strip; one PSUM bank accumulates all functionals over the whole
  kernel (start on the very first warmup matmul, stop on the last real
  one).  48 warmup matmuls through an all-zero selector window keep the
  PE HAM clock-gate at 8/8 before real work lands (a LDWEIGHTS between
  accumulating matmuls forces isolated fill+drain, ~414ns per 512-col
  matmul instead of 216).
  Fused/dummy outputs use stride-0 broadcast APs (engines tolerate write
  collisions; saves 24KB SBUF).
  Final: PSUM [32,512] -> SBUF -> DRAM; accums -> DRAM; host recovers R_l,
  applies exact count corrections and the dice formula in float64.
"""

import numpy as np

# ---- hardcoded problem geometry -------------------------------------------
B = 8                      # batch == number of cores
P = 128                    # SBUF partitions
F = 16384                  # free dim per core (128*128*128 / 128)
N = P * F                  # elements per core
NB = 10                    # labels 1..10 (0 = background)
STAGE = 2048               # DMA staging columns
UCOLS = 4096               # u-tile columns per super-chunk
NSUPER = F // UCOLS        # 4
NCH = UCOLS // STAGE       # 2 staging chunks per super
PE_CHUNK = 512             # matmul moving free dim
N_WARMUP = 24              # PE warmup matmuls
EPS = 1e-6

# Functional assignment (fixed across supers).  All PE-path funcs are
# DMA-folded (4096->2048) then reduced by 4 matmuls.  Ordered by stream
# readiness (u_t completes first, then u_x, then u_xt).
# direct: TS -> 8 matmuls.  fold: TS pairs share a [P, 2*UCOLS] slab; ONE
# strided accumulating DMA folds both (halves the Q7 SWDGE cost), then 4
# matmuls each.
DIRECT_FUNCS = [("ut", 10), ("ux", 1), ("ux", 2), ("ux", 3), ("ux", 4)]
FOLD_FUNCS = (
    [("ux", l) for l in range(5, 11)]
    + [("uxt", l) for l in range(1, 9)]
)  # 14 -> 7 pairs
PE_FUNCS = DIRECT_FUNCS + FOLD_FUNCS  # PSUM row j = index; 19 rows, max-form
ACT_FUNCS = (
    [("ut", l) for l in range(1, 10)] + [("uxt", 9), ("uxt", 10)]
)  # relu-form, 11
N_EARLY_HALVES = 5         # super-0: direct funcs emitted as two halves
PE_LAG = 3                 # funcs between production and their matmuls

NROW = 32                  # SEL width / psum partition rows

_CACHE = {}


def _build_program():
    import concourse.mybir as mybir
    from concourse import bacc, tile

    fp32 = mybir.dt.float32
    fp16 = mybir.dt.float16
    int32 = mybir.dt.int32
    Alu = mybir.AluOpType
    Act = mybir.ActivationFunctionType

    nc = bacc.Bacc("TRN2", target_bir_lowering=False, debug=False)

    # activation(bias=float) needs a registered const AP per value
    for l in range(1, 11):
        val = float(-l)
        th = nc.alloc_sbuf_tensor(f"const-float32--{l}", [128, 1], fp32)
        nc.gpsimd.memset(th.ap(), val)
        nc.const_aps.aps[(fp32, val)] = th.ap()
    nc.all_engine_barrier()

    x_d = nc.dram_tensor("x", [P, F], fp32, kind="ExternalInput").ap()
    t_d = nc.dram_tensor("t", [P, F], fp32, kind="ExternalInput").ap()
    s_d = nc.dram_tensor("s", [P, F], int32, kind="ExternalInput").ap()

    n_acc_cols = len(ACT_FUNCS) * NSUPER
    acc_d = nc.dram_tensor("acc", [P, n_acc_cols], fp32, kind="ExternalOutput").ap()
    pe_d = nc.dram_tensor("pe", [NROW, PE_CHUNK], fp32, kind="ExternalOutput").ap()

    # matmul count: direct funcs reduce UCOLS cols, folded funcs UCOLS/2
    n_mm = NSUPER * (
        len(DIRECT_FUNCS) * (UCOLS // PE_CHUNK)
        + len(FOLD_FUNCS) * (UCOLS // 2 // PE_CHUNK)
    )

    with tile.TileContext(nc) as tc:
        with (
            tc.tile_pool(name="io", bufs=2) as io_pool,
            tc.tile_pool(name="tr", bufs=2) as tr_pool,
            tc.tile_pool(name="up", bufs=2) as u_pool,
            tc.tile_pool(name="scr", bufs=3) as scr_pool,
            tc.tile_pool(name="slab", bufs=3) as slab_pool,
            tc.tile_pool(name="persist", bufs=1) as pp,
            tc.tile_pool(name="psum", bufs=1, space="PSUM") as psp,
        ):
            # SEL strip: ones at column 32; SEL_j = strip[:, 32-j : 64-j].
            # strip[:, 0:32] is an all-zero selector (warmup).
            strip = pp.tile([P, 64], fp16, tag="strip")
            nc.vector.memset(strip[:], 0.0)
            nc.vector.memset(strip[:, 32:33], 1.0)

            acc_f = pp.tile([P, n_acc_cols], fp32, tag="acc_f")
            # stride-0 dummy output for fused ACT ramps (never read)
            dummy = pp.tile([P, 1], fp16, tag="dummy")
            dummy_bcast = dummy[:, 0:1].broadcast_to((P, UCOLS))
            scr_w = pp.tile([P, PE_CHUNK], fp16, tag="scr_w")
            nc.vector.memset(scr_w[:], 0.0)
            psum = psp.tile([NROW, PE_CHUNK], fp32, tag="psum")

            # PE warmup: zero-selector matmuls hold the HAM clock warm and
            # initialize PSUM (first has start=True).
            sel0 = strip[:, 0:32]
            for w in range(N_WARMUP):
                nc.tensor.matmul(
                    psum[:], sel0, scr_w[:],
                    start=(w == 0), stop=False, skip_group_check=True,
                )

            mm_idx = 0

            def pe_reduce(j, scr, ncols):
                nonlocal mm_idx
                sel = strip[:, 32 - j : 64 - j]
                for c in range(ncols // PE_CHUNK):
                    mm_idx += 1
                    nc.tensor.matmul(
                        psum[:],
                        sel,
                        scr[:, c * PE_CHUNK : (c + 1) * PE_CHUNK],
                        start=False,
                        stop=(mm_idx == n_mm),
                        skip_group_check=True,
                    )

            acc_col = 0

            def fused_col():
                nonlocal acc_col
                c = acc_col
                acc_col += 1
                return acc_f[:, c : c + 1]

            u_sets = [None] * NSUPER

            def alloc_u(si):
                u_x = u_pool.tile([P, UCOLS], fp16, tag="u_x")
                u_t = u_pool.tile([P, UCOLS], fp16, tag="u_t")
                u_xt = u_pool.tile([P, UCOLS], fp16, tag="u_xt")
                u_sets[si] = {"ux": u_x, "ut": u_t, "uxt": u_xt}

            def build_thunks(si):
                """One thunk = one DVE build op; the first thunk of each
                chunk also issues that chunk's DMAs."""
                srcs = u_sets[si]
                thunks = []
                for c in range(NCH):
                    ci = si * NCH + c
                    sl = slice(ci * STAGE, (ci + 1) * STAGE)
                    hsl = slice(c * STAGE, (c + 1) * STAGE)
                    state = {}

                    def dma_and_s16(sl=sl, state=state):
                        x_c = io_pool.tile([P, STAGE], fp32, tag="x_c")
                        t_c = io_pool.tile([P, STAGE], fp32, tag="t_c")
                        s_c = io_pool.tile([P, STAGE], int32, tag="s_c")
                        nc.sync.dma_start(out=x_c[:], in_=x_d[:, sl])
                        nc.sync.dma_start(out=t_c[:], in_=t_d[:, sl])
                        nc.sync.dma_start(out=s_c[:], in_=s_d[:, sl])
                        s16 = tr_pool.tile([P, STAGE], fp16, tag="s16")
                        t16 = tr_pool.tile([P, STAGE], fp16, tag="t16")
                        x16 = tr_pool.tile([P, STAGE], fp16, tag="x16")
                        xt16 = tr_pool.tile([P, STAGE], fp16, tag="xt16")
                        state.update(x_c=x_c, t_c=t_c, s_c=s_c, s16=s16,
                                     t16=t16, x16=x16, xt16=xt16)
                        nc.vector.tensor_copy(s16[:], s_c[:])

                    thunks.append(dma_and_s16)
                    thunks.append(lambda st=state: nc.vector.tensor_copy(
                        st["t16"][:], st["t_c"][:]))
                    thunks.append(lambda st=state: nc.vector.tensor_copy(
                        st["x16"][:], st["x_c"][:]))
                    thunks.append(lambda st=state, h=hsl: nc.vector.tensor_tensor(
                        srcs["ut"][:, h], st["t16"][:], st["s16"][:], Alu.add))
                    thunks.append(lambda st=state, h=hsl: nc.vector.tensor_tensor(
                        srcs["ux"][:, h], st["x16"][:], st["s16"][:], Alu.add))
                    thunks.append(lambda st=state: nc.vector.tensor_tensor(
                        st["xt16"][:], st["x16"][:], st["t16"][:], Alu.mult))
                    thunks.append(lambda st=state, h=hsl: nc.vector.tensor_tensor(
                        srcs["uxt"][:, h], st["xt16"][:], st["s16"][:], Alu.add))
                return thunks

            pending = []  # lag queue of (j, scr_ap, folded_cols)

            def flush_pending(keep):
                while len(pending) > keep:
                    j, scr, ncols = pending.pop(0)
                    pe_reduce(j, scr, ncols)

            def ramp_thunks(si):
                srcs = u_sets[si]
                thunks = []
                # direct funcs (halved in super 0 for early PE work)
                for j, (kind, l) in enumerate(DIRECT_FUNCS):
                    ranges = (
                        [(0, UCOLS // 2), (UCOLS // 2, UCOLS)]
                        if si == 0 else [(0, UCOLS)]
                    )
                    for lo, hi in ranges:
                        def ts(j=j, kind=kind, l=l, lo=lo, hi=hi):
                            w = hi - lo
                            scr = scr_pool.tile([P, w], fp16, tag="scrd")
                            nc.vector.tensor_scalar(
                                scr[:], srcs[kind][:, lo:hi], float(l),
                                None, Alu.max
                            )
                            pending.append((j, scr[:], w))
                            flush_pending(PE_LAG)
                        thunks.append(ts)
                # folded funcs, in pairs sharing one slab + one DMA
                for pi in range(0, len(FOLD_FUNCS), 2):
                    jA = len(DIRECT_FUNCS) + pi
                    jB = jA + 1
                    (kA, lA), (kB, lB) = FOLD_FUNCS[pi], FOLD_FUNCS[pi + 1]

                    pair_state = {}

                    def tsA(kA=kA, lA=lA, st=pair_state):
                        slab = slab_pool.tile([P, 2 * UCOLS], fp16, tag="slab")
                        st["slab"] = slab
                        nc.vector.tensor_scalar(
                            slab[:, 0:UCOLS], srcs[kA][:], float(lA),
                            None, Alu.max
                        )

                    def tsB(kB=kB, lB=lB, jA=jA, jB=jB, st=pair_state):
                        slab = st["slab"]
                        nc.vector.tensor_scalar(
                            slab[:, UCOLS : 2 * UCOLS], srcs[kB][:],
                            float(lB), None, Alu.max
                        )
                        H = UCOLS // 2
                        v = slab[:].rearrange("p (a b) -> p a b", a=4)
                        # fold both funcs with one strided accumulating DMA:
                        # slab[0:H] += slab[H:2H]; slab[2H:3H] += slab[3H:4H]
                        nc.gpsimd.dma_start(
                            out=v[:, 0::2, :],
                            in_=v[:, 1::2, :],
                            accum_op=Alu.add,
                        )
                        pending.append((jA, slab[:, 0:H], H))
                        pending.append((jB, slab[:, 2 * H : 3 * H], H))
                        flush_pending(PE_LAG)

                    thunks.append(tsA)
                    thunks.append(tsB)
                return thunks

            # super 0 builds upfront
            alloc_u(0)
            for th in build_thunks(0):
                th()

            for si in range(NSUPER):
                srcs = u_sets[si]
                for kind, l in ACT_FUNCS:
                    nc.scalar.activation(
                        dummy_bcast, srcs[kind][:], Act.Relu,
                        bias=float(-l), scale=1.0,
                        accum_out=fused_col(),
                    )
                # DVE: ramps of si interleaved with builds of si+1
                nxt = []
                if si + 1 < NSUPER:
                    alloc_u(si + 1)
                    nxt = build_thunks(si + 1)
                ramps = ramp_thunks(si)
                ri = bi = 0
                while ri < len(ramps) or bi < len(nxt):
                    if ri < len(ramps):
                        ramps[ri]()
                        ri += 1
                    if bi < len(nxt):
                        nxt[bi]()
                        bi += 1

            flush_pending(0)
            assert mm_idx == n_mm
            pe_sb = pp.tile([NROW, PE_CHUNK], fp32, tag="pe_sb")
            nc.vector.tensor_copy(pe_sb[:], psum[:])
            nc.sync.dma_start(out=pe_d[:], in_=pe_sb[:])
            nc.sync.dma_start(out=acc_d[:], in_=acc_f[:])

    nc.compile()
    return nc


def _get_program():
    if "nc" not in _CACHE:
        _CACHE["nc"] = _build_program()
    return _CACHE["nc"]


def _recover_sums(acc, pe, Cge):
    """acc: [P, n_acc_cols] fp32; pe: [NROW, PE_CHUNK] fp32; Cge: exact
    C_{>=l} counts (len 13).  Returns S[kind][l] for l=1..10."""
    R = {v: np.zeros(12) for v in ("ux", "ut", "uxt")}
    accs = acc.astype(np.float64)
    col = 0
    for si in range(NSUPER):
        for kind, l in ACT_FUNCS:       # relu-form: R_l directly
            R[kind][l] += accs[:, col].sum()
            col += 1
    pes = pe.astype(np.float64)
    for j, (kind, l) in enumerate(PE_FUNCS):  # max-form, full stream
        R[kind][l] += pes[j].sum() - l * N

    S = {}
    for v in ("ux", "ut", "uxt"):
        Sv = np.zeros(11)
        for l in range(1, 11):
            Rl1 = R[v][l + 1] if l + 1 <= 10 else 0.0
            Sv[l] = R[v][l] - Rl1 - Cge[l + 1]
        S[v] = Sv
    return S


def kernel(input, target, block):
    from concourse.bass_utils import run_bass_kernel_spmd

    nc = _get_program()

    in_maps = []
    for b in range(B):
        in_maps.append(
            {
                "x": np.ascontiguousarray(input[b].reshape(P, F)),
                "t": np.ascontiguousarray(target[b].reshape(P, F)),
                "s": np.ascontiguousarray(block[b].reshape(P, F)),
            }
        )
    res = run_bass_kernel_spmd(nc, in_maps, list(range(B))).results

    intersect = np.zeros((B, NB))
    input_area = np.zeros((B, NB))
    target_area = np.zeros((B, NB))
    counts = np.zeros((B, NB))
    for b in range(B):
        cnt = np.bincount(block[b].reshape(-1), minlength=12)[:12].astype(np.float64)
        Cge = np.concatenate([np.cumsum(cnt[::-1])[::-1], [0.0]])  # C_{>=l}, l=0..12
        S = _recover_sums(res[b]["acc"], res[b]["pe"], Cge)
        input_area[b] = S["ux"][1:11]
        target_area[b] = S["ut"][1:11]
        intersect[b] = S["uxt"][1:11]
        counts[b] = cnt[1:11]

    # dice combination (mirror reference, float64; empty-segment test uses
    # exact integer counts, equivalent to target_area == 0 for this data)
    empty = counts == 0
    denom = input_area + target_area + 2.0 * EPS
    batch_loss = 1.0 - 2.0 * intersect / denom
    batch_loss = np.where(empty, 0.0, batch_loss)
    valid = (~empty).sum(axis=0).astype(np.float64)
    loss_per_block = batch_loss.sum(axis=0) / np.maximum(valid, 1.0)

    present = counts.sum(axis=0) > 0
    num = present.sum()
    loss = np.where(present, loss_per_block, 0.0).sum() / num
    return (np.float32(loss), 0)
